# revision 1
# baseline (speedup 1.0000x reference)
"""Trainium2 Bass kernel for a single decoder block (B=2, T=2048, C=1024,
NH=16, DFF=4096), distributed over 8 NeuronCores.

Sharding: token-parallel. Core c owns tokens [512*(c%4), 512*(c%4+1)) of
batch c//4. Each core computes q/k/v for its own tokens, AllGathers k and v
within its 4-core batch group, runs causal attention for all 16 heads over
its query chunk (full key range, causality enforced by multiplicative
masks), then LayerNorm residual + FFN + final LayerNorm residual locally.
The host reassembles the 8 output chunks.

Activations flow feature-major ([feature, token]) so matmul contractions
always have the contraction dim on SBUF partitions; fp32r (TF32-like) is
used for all matmul operands, fp32 elsewhere.
"""

import sys

if "/opt/trn_rl_repo" not in sys.path:
    sys.path.insert(0, "/opt/trn_rl_repo")

from contextlib import ExitStack

import numpy as np
import ml_dtypes

B, T, C = 2, 2048, 1024
NH, HD, DFF = 16, 64, 4096
N_CORES = 8
TCH = 512          # tokens per core
NCT = C // 128     # 8 feature tiles
NKT = T // 128     # 16 key tiles per batch
NPAIR = NH // 2    # 8 head pairs
SCALE = 1.0 / 32.0  # 1/sqrt(C)
EPS = 1e-5

_CACHE = {}


def _build(reps=1, collective=True):
    import concourse.mybir as mybir
    import concourse.tile as tile
    from concourse import bacc

    F32 = mybir.dt.float32
    F32R = mybir.dt.float32r
    BF16 = mybir.dt.bfloat16
    AF = mybir.ActivationFunctionType
    ALU = mybir.AluOpType

    nc = bacc.Bacc("TRN2", target_bir_lowering=False, debug=False,
                   num_devices=N_CORES)

    def din(name, shape):
        return nc.dram_tensor(name, shape, F32, kind="ExternalInput").ap()

    def dinb(name, shape):
        return nc.dram_tensor(name, shape, mybir.dt.bfloat16,
                              kind="ExternalInput").ap()

    xct = dinb("xct", [C, TCH])
    wq = dinb("wq", [C, C])
    wk = dinb("wk", [C, C])
    wv = dinb("wv", [C, C])
    w1 = dinb("w1", [C, DFF])
    w2 = dinb("w2", [DFF, C])
    bq_r = din("bq_r", [128, NCT])
    bk_r = din("bk_r", [128, NCT])
    bv_row = din("bv_row", [1, C])
    b1_r = din("b1_r", [128, DFF // 128])
    b2_r = din("b2_r", [128, NCT])
    lnw_r = din("lnw_r", [128, NCT])
    lnw_row = din("lnw_row", [1, C])
    ident_in = din("ident_in", [128, 128])
    ones_in = din("ones_in", [128, 128])
    maskq = nc.dram_tensor("maskq", [NKT, 128, TCH], mybir.dt.bfloat16,
                           kind="ExternalInput").ap()
    outc = nc.dram_tensor("outc", [TCH, C], F32, kind="ExternalOutput").ap()

    import concourse.bass as bass

    def bcast_row(row_ap, parts=128):
        # view a [1, N] DRAM row as [parts, N] with partition step 0
        return bass.AP(tensor=row_ap.tensor, offset=row_ap.offset,
                       ap=[[0, parts]] + list(row_ap.ap[1:]))

    with tile.TileContext(nc) as tc, ExitStack() as S0, \
            nc.allow_low_precision(reason="fp32r matmul operand rounding"):
      persist = S0.enter_context(tc.tile_pool(name="persist", bufs=1))
      dram = S0.enter_context(tc.tile_pool(name="dram", bufs=1, space="DRAM"))
      for _rep in range(reps):

          # ---- phase-1-critical constants only; the rest load later
          bk_sb = persist.tile([128, NCT], F32)
          nc.sync.dma_start(bk_sb, bk_r)
          bq_sb = persist.tile([128, NCT], F32)
          nc.sync.dma_start(bq_sb, bq_r)
          ones_all = persist.tile([128, 128], F32R)
          nc.sync.dma_start(ones_all, ones_in.bitcast(F32R))
          eps_sb = persist.tile([128, 1], F32)
          nc.vector.memset(eps_sb, EPS)
          ident_f = persist.tile([128, 128], F32)
          bv_sb = persist.tile([128, C], F32)
          lnwbc_sb = persist.tile([128, C], F32)
          b1_sb = persist.tile([128, DFF // 128], F32)
          b2_sb = persist.tile([128, NCT], F32)
          lnw_sb = persist.tile([128, NCT], F32)

          # ---- DRAM buffers for the k/v AllGathers (v first, k second)
          VROW = NH * (HD + 1)  # 1040: per-token v row with ones col per head
          kg_loc = dram.tile([C * TCH], BF16)
          vg_loc = dram.tile([TCH * VROW], BF16)
          kg_out = dram.tile([4, C * TCH], BF16)
          vg_out = dram.tile([4, TCH * VROW], BF16)
          k_loc = kg_loc[:].rearrange("(f t) -> f t", t=TCH)   # [1024, 512]
          v_loc = vg_loc[:].rearrange("(t x) -> t x", x=VROW)  # [512, 1040]

          with ExitStack() as SQA:
              qa_pool = SQA.enter_context(tc.tile_pool(name="qa", bufs=1))
              qT = qa_pool.tile([128, NPAIR, TCH], BF16)
              aT = qa_pool.tile([128, NCT, TCH], F32R)
              w1p = SQA.enter_context(tc.tile_pool(name="w1p", bufs=3))
              w2p = SQA.enter_context(tc.tile_pool(name="w2p", bufs=2))
              SATT = SQA.enter_context(ExitStack())
              mpool = SATT.enter_context(tc.tile_pool(name="mpool", bufs=1))
              vaugp = SATT.enter_context(tc.tile_pool(name="vaugp", bufs=1))
              kpool = SATT.enter_context(tc.tile_pool(name="kpool", bufs=6))
              masks_sb = mpool.tile([128, NKT, TCH], BF16)

              # ================= Phase 1: x^T, q/k/v projections ============
              with ExitStack() as S1:
                  xtp = S1.enter_context(tc.tile_pool(name="xtp", bufs=1))
                  wqk = S1.enter_context(tc.tile_pool(name="wqk", bufs=3))
                  wvp = S1.enter_context(tc.tile_pool(name="wvp", bufs=1))
                  kvsb = S1.enter_context(tc.tile_pool(name="kvsb", bufs=4))
                  vstgp = S1.enter_context(tc.tile_pool(name="vstgp", bufs=1))
                  qkps = S1.enter_context(tc.tile_pool(name="qkps", bufs=4, space="PSUM"))
                  vps = S1.enter_context(tc.tile_pool(name="vps", bufs=4, space="PSUM"))

                  xT = xtp.tile([128, NCT, TCH], BF16)
                  nc.sync.dma_start(
                      xT, xct.rearrange("(ci p) t -> p ci t", p=128))

                  # k^T first: it gates the k AllGather and S^T
                  for p in range(NCT):
                      wt = wqk.tile([128, NCT, 128], BF16, tag="wt")
                      nc.sync.dma_start(
                          wt, wk.rearrange("(ci r) f -> r ci f", r=128)
                          [:, :, p * 128:(p + 1) * 128])
                      ps = qkps.tile([128, TCH], F32, tag="qkp")
                      for ci in range(NCT):
                          nc.tensor.matmul(ps, wt[:, ci, :], xT[:, ci, :],
                                           start=(ci == 0), stop=(ci == NCT - 1))
                      ksb = kvsb.tile([128, TCH], BF16, tag="ksb")
                      nc.scalar.activation(ksb, ps, AF.Identity,
                                           bias=bk_sb[:, p:p + 1])
                      nc.sync.dma_start(k_loc[p * 128:(p + 1) * 128, :], ksb)
                  if collective:
                      nc.gpsimd.collective_compute(
                          "AllGather", mybir.AluOpType.bypass,
                          replica_groups=[[0, 1, 2, 3], [4, 5, 6, 7]],
                          ins=[kg_loc[:].opt()], outs=[kg_out[:].opt()])
                  else:
                      for g in range(4):
                          nc.sync.dma_start(kg_out[g, 0:1024], kg_loc[0:1024])

                  nc.gpsimd.dma_start(bv_sb, bcast_row(bv_row))

                  # v: token-major, written directly in v_aug layout
                  # [512 tok, 16 head, 64+1] with a ones column per head
                  vstg = [vstgp.tile([128, NH, HD + 1], BF16, name=f"vstg{tt}")
                          for tt in range(4)]
                  for tt in range(4):
                      nc.vector.memset(vstg[tt][:, :, HD:HD + 1], 1.0)
                  for fvt in range(2):
                      wt = wvp.tile([128, NCT, TCH], BF16, tag="wtv")
                      nc.sync.dma_start(
                          wt, wv.rearrange("(ci r) f -> r ci f", r=128)
                          [:, :, fvt * TCH:(fvt + 1) * TCH])
                      pss = [vps.tile([128, TCH], F32, tag="vp", name=f"vp{fvt}_{i}")
                             for i in range(4)]
                      for ci in range(NCT):
                          for tt in range(4):
                              nc.tensor.matmul(
                                  pss[tt], xT[:, ci, tt * 128:(tt + 1) * 128],
                                  wt[:, ci, :],
                                  start=(ci == 0), stop=(ci == NCT - 1))
                      bvv = bv_sb[:, fvt * TCH:(fvt + 1) * TCH].rearrange(
                          "p (h d) -> p h d", d=HD)
                      for tt in range(4):
                          nc.vector.tensor_add(
                              vstg[tt][:, 8 * fvt:8 * fvt + 8, 0:HD],
                              pss[tt][:, :].rearrange("p (h d) -> p h d", d=HD),
                              bvv)
                  for tt in range(4):
                      nc.sync.dma_start(
                          v_loc[tt * 128:(tt + 1) * 128, :],
                          vstg[tt][:, :, :])
                  if collective:
                      nc.gpsimd.collective_compute(
                          "AllGather", mybir.AluOpType.bypass,
                          replica_groups=[[0, 1, 2, 3], [4, 5, 6, 7]],
                          ins=[vg_loc[:].opt()], outs=[vg_out[:].opt()])
                  else:
                      for g in range(4):
                          nc.sync.dma_start(vg_out[g, 0:1024], vg_loc[0:1024])

                  nc.sync.dma_start(
                      masks_sb, maskq.rearrange("k p t -> p k t"))
                  nc.sync.dma_start(ident_f, ident_in)
                  nc.gpsimd.dma_start(lnwbc_sb, bcast_row(lnw_row))
                  nc.sync.dma_start(b1_sb, b1_r)
                  nc.sync.dma_start(b2_sb, b2_r)
                  nc.sync.dma_start(lnw_sb, lnw_r)

                  # q^T: [1024 feat, 512 tok], feature-major
                  for p in range(NCT):
                      wt = wqk.tile([128, NCT, 128], BF16, tag="wt")
                      nc.sync.dma_start(
                          wt, wq.rearrange("(ci r) f -> r ci f", r=128)
                          [:, :, p * 128:(p + 1) * 128])
                      ps = qkps.tile([128, TCH], F32, tag="qkp")
                      for ci in range(NCT):
                          nc.tensor.matmul(ps, wt[:, ci, :], xT[:, ci, :],
                                           start=(ci == 0), stop=(ci == NCT - 1))
                      nc.scalar.activation(qT[:, p, :], ps, AF.Identity,
                                           bias=bq_sb[:, p:p + 1])

              # ================= Phase 3: attention =========================
              with ExitStack() as S3:
                  pup = S3.enter_context(tc.tile_pool(name="pup", bufs=4))
                  dnp = S3.enter_context(tc.tile_pool(name="dnp", bufs=3))
                  stps = S3.enter_context(tc.tile_pool(name="stps", bufs=2, space="PSUM"))
                  avps = S3.enter_context(tc.tile_pool(name="avps", bufs=3, space="PSUM"))
                  rdps = S3.enter_context(tc.tile_pool(name="rdps", bufs=1, space="PSUM"))

                  # v_aug[j, head, 0:64] = v_head, [.., 64] = 1.0 (denominator)
                  vaug = vaugp.tile([128, NKT, NH, HD + 1], BF16)
                  for jt in range(NKT):
                      r, u = divmod(jt, 4)
                      vsrc = vg_out[r].rearrange("(t x) -> t x", x=VROW)
                      nc.sync.dma_start(
                          vaug[:, jt, :, :].rearrange("p h d -> p (h d)"),
                          vsrc[u * 128:(u + 1) * 128, :])

                  for p in range(NPAIR):
                      avA = avps.tile([HD + 1, TCH], F32, tag="av")
                      avB = avps.tile([HD + 1, TCH], F32, tag="av")
                      ktile = None
                      for kt in range(NKT):
                          r, u = divmod(kt, 4)
                          if u == 0:
                              ktile = kpool.tile([128, TCH], BF16, tag="kt")
                              ksrc = kg_out[r].rearrange("(f t) -> f t", t=TCH)
                              nc.sync.dma_start(
                                  ktile, ksrc[p * 128:(p + 1) * 128, :])
                          st = stps.tile([128, 2, TCH], F32, tag="st")
                          nc.tensor.matmul(st[:, 0, :],
                                           ktile[0:64, u * 128:(u + 1) * 128],
                                           qT[0:64, p, :], start=True, stop=True)
                          nc.tensor.matmul(st[:, 1, :],
                                           ktile[64:128, u * 128:(u + 1) * 128],
                                           qT[64:128, p, :], start=True, stop=True)
                          pu = pup.tile([128, 2, TCH], BF16, tag="pu")
                          nc.scalar.activation(pu[:], st[:], AF.Exp, scale=SCALE)
                          m = masks_sb[:, kt, :]
                          m2 = bass.AP(tensor=m.tensor, offset=m.offset,
                                       ap=[list(m.ap[0]), [0, 2],
                                           list(m.ap[1])])
                          nc.vector.tensor_mul(pu[:, :, :], pu[:, :, :], m2)
                          nc.tensor.matmul(avA, vaug[:, kt, 2 * p, :],
                                           pu[:, 0, :],
                                           start=(kt == 0), stop=(kt == NKT - 1))
                          nc.tensor.matmul(avB, vaug[:, kt, 2 * p + 1, :],
                                           pu[:, 1, :],
                                           start=(kt == 0), stop=(kt == NKT - 1))
                      # normalize: a_head = av[0:64] / av[64]
                      for av, odd in ((avA, 0), (avB, 1)):
                          den = dnp.tile([128, TCH], F32R, tag="den")
                          nc.vector.tensor_copy(den[64:65, :], av[64:65, :])
                          rd = rdps.tile([64, TCH], F32, tag="rd")
                          nc.tensor.matmul(rd, ones_all[64:65, 0:64],
                                           den[64:65, :], start=True, stop=True)
                          rdsb = dnp.tile([64, TCH], F32, tag="rdsb")
                          nc.vector.reciprocal(rdsb[:], rd[:])
                          if not odd:
                              nc.vector.tensor_mul(aT[0:64, p, :], av[0:64, :],
                                                   rdsb[:])
                          else:
                              tmp = dnp.tile([64, TCH], F32R, tag="tmp")
                              nc.vector.tensor_mul(tmp[:], av[0:64, :], rdsb[:])
                              nc.sync.dma_start(aT[64:128, p, :], tmp[:])

              SATT.close()

              # ================= Phase 4: h = a + LN(a), feature-major ======
              with ExitStack() as SH:
                  hp = SH.enter_context(tc.tile_pool(name="hp", bufs=1))
                  hT = hp.tile([128, NCT, TCH], BF16)
                  fT = hp.tile([128, NCT, TCH], F32)

                  with ExitStack() as S4:
                      sqp = S4.enter_context(tc.tile_pool(name="sqp", bufs=3))
                      stsb = S4.enter_context(tc.tile_pool(name="stsb", bufs=1))
                      smps = S4.enter_context(tc.tile_pool(name="smps", bufs=2, space="PSUM"))
                      bcps = S4.enter_context(tc.tile_pool(name="bcps", bufs=2, space="PSUM"))

                      sum_ps = smps.tile([1, TCH], F32, tag="sm")
                      sq_ps = smps.tile([1, TCH], F32, tag="sm")
                      for ci in range(NCT):
                          nc.tensor.matmul(sum_ps, ones_all[:, 0:1], aT[:, ci, :],
                                           start=(ci == 0), stop=(ci == NCT - 1))
                      for ci in range(NCT):
                          asq = sqp.tile([128, TCH], F32R, tag="asq")
                          nc.scalar.activation(asq, aT[:, ci, :], AF.Square)
                          nc.tensor.matmul(sq_ps, ones_all[:, 0:1], asq[:],
                                           start=(ci == 0), stop=(ci == NCT - 1))
                      mu_sb = stsb.tile([1, TCH], F32R, tag="s1")
                      nc.vector.tensor_scalar_mul(mu_sb, sum_ps, 1.0 / C)
                      ex2 = stsb.tile([1, TCH], F32, tag="s2")
                      nc.vector.tensor_scalar_mul(ex2, sq_ps, 1.0 / C)
                      musq = stsb.tile([1, TCH], F32, tag="s3")
                      nc.vector.tensor_mul(musq, mu_sb, mu_sb)
                      var = stsb.tile([1, TCH], F32, tag="s4")
                      nc.vector.tensor_sub(var, ex2, musq)
                      sd = stsb.tile([1, TCH], F32, tag="s5")
                      nc.scalar.activation(sd, var, AF.Sqrt, bias=eps_sb[0:1, :])
                      rs_sb = stsb.tile([1, TCH], F32R, tag="s6")
                      nc.vector.reciprocal(rs_sb, sd)
                      mu_bc = bcps.tile([128, TCH], F32, tag="bc")
                      nc.tensor.matmul(mu_bc, ones_all[0:1, :], mu_sb[:],
                                       start=True, stop=True)
                      rs_bc = bcps.tile([128, TCH], F32, tag="bc")
                      nc.tensor.matmul(rs_bc, ones_all[0:1, :], rs_sb[:],
                                       start=True, stop=True)
                      for ci in range(NCT):
                          t1 = sqp.tile([128, TCH], F32, tag="t1")
                          nc.vector.tensor_sub(t1, aT[:, ci, :], mu_bc)
                          t2 = sqp.tile([128, TCH], F32, tag="t2")
                          nc.vector.tensor_mul(t2, t1, rs_bc)
                          nc.vector.scalar_tensor_tensor(
                              out=hT[:, ci, :], in0=t2,
                              scalar=lnw_sb[:, ci:ci + 1], in1=aT[:, ci, :],
                              op0=ALU.mult, op1=ALU.add)

                  # ================= Phase 5/6: FFN =========================
                  with ExitStack() as S5:
                      gp = S5.enter_context(tc.tile_pool(name="gp", bufs=1))
                      ffps = S5.enter_context(tc.tile_pool(name="ffps", bufs=4, space="PSUM"))

                      gT = gp.tile([128, DFF // 128, TCH], BF16)
                      for mt in range(DFF // 128):
                          wt = w1p.tile([128, NCT, 128], BF16, tag="w1t")
                          nc.sync.dma_start(
                              wt, w1.rearrange("(ci r) f -> r ci f", r=128)
                              [:, :, mt * 128:(mt + 1) * 128])
                          ps = ffps.tile([128, TCH], F32, tag="f1")
                          for ci in range(NCT):
                              nc.tensor.matmul(ps, wt[:, ci, :], hT[:, ci, :],
                                               start=(ci == 0), stop=(ci == NCT - 1))
                          nc.scalar.activation(gT[:, mt, :], ps, AF.Relu,
                                               bias=b1_sb[:, mt:mt + 1])
                      for ci in range(NCT):
                          wt = w2p.tile([128, DFF // 128, 128], BF16, tag="w2t")
                          nc.sync.dma_start(
                              wt, w2.rearrange("(gk r) f -> r gk f", r=128)
                              [:, :, ci * 128:(ci + 1) * 128])
                          ps = ffps.tile([128, TCH], F32, tag="f2")
                          for gk in range(DFF // 128):
                              nc.tensor.matmul(ps, wt[:, gk, :], gT[:, gk, :],
                                               start=(gk == 0),
                                               stop=(gk == DFF // 128 - 1))
                          nc.scalar.activation(fT[:, ci, :], ps, AF.Identity,
                                               bias=b2_sb[:, ci:ci + 1])

                  # ================= Phase 7: out = f + LN(f), token-major ==
                  with ExitStack() as S7:
                      op7 = S7.enter_context(tc.tile_pool(name="op7", bufs=2))
                      tp7 = S7.enter_context(tc.tile_pool(name="tp7", bufs=2, space="PSUM"))

                      for tt in range(4):
                          ftok = op7.tile([128, C], F32, tag="ftok")
                          for ci in range(NCT):
                              tp = tp7.tile([128, 128], F32, tag="tp")
                              nc.tensor.transpose(
                                  tp, fT[:, ci, tt * 128:(tt + 1) * 128],
                                  ident_f[:])
                              nc.vector.tensor_copy(
                                  ftok[:, ci * 128:(ci + 1) * 128], tp[:])
                          stats = op7.tile([128, 2, nc.vector.BN_STATS_DIM],
                                           F32, tag="bst")
                          nc.vector.bn_stats(stats[:, 0], ftok[:, 0:512])
                          nc.vector.bn_stats(stats[:, 1], ftok[:, 512:1024])
                          mv = op7.tile([128, nc.vector.BN_AGGR_DIM], F32,
                                        tag="mv")
                          nc.vector.bn_aggr(mv, stats)
                          rs7 = op7.tile([128, 1], F32, tag="rs7")
                          nc.scalar.activation(rs7, mv[:, 1:2], AF.Sqrt,
                                               bias=eps_sb)
                          nc.vector.reciprocal(rs7, rs7)
                          t1 = op7.tile([128, C], F32, tag="t17")
                          nc.vector.tensor_scalar(
                              out=t1, in0=ftok, scalar1=mv[:, 0:1], scalar2=rs7,
                              op0=ALU.subtract, op1=ALU.mult)
                          nc.vector.tensor_mul(t1, t1, lnwbc_sb)
                          otok = op7.tile([128, C], F32, tag="otok")
                          nc.vector.tensor_add(otok, t1, ftok)
                          nc.sync.dma_start(outc[tt * 128:(tt + 1) * 128, :],
                                            otok)

    nc.compile()
    return nc


def _stage(inputs):
    x = np.ascontiguousarray(np.asarray(inputs["x"], dtype=np.float32))
    bf = ml_dtypes.bfloat16
    shared = {
        "wq": np.ascontiguousarray(np.asarray(inputs["Wq"], np.float32).astype(bf)),
        "wk": np.ascontiguousarray(np.asarray(inputs["Wk"], np.float32).astype(bf)),
        "wv": np.ascontiguousarray(np.asarray(inputs["Wv"], np.float32).astype(bf)),
        "w1": np.ascontiguousarray(np.asarray(inputs["W1"], np.float32).astype(bf)),
        "w2": np.ascontiguousarray(np.asarray(inputs["W2"], np.float32).astype(bf)),
        "bq_r": np.ascontiguousarray(
            np.asarray(inputs["bq"], np.float32).reshape(NCT, 128).T),
        "bk_r": np.ascontiguousarray(
            np.asarray(inputs["bk"], np.float32).reshape(NCT, 128).T),
        "bv_row": np.ascontiguousarray(
            np.asarray(inputs["bv"], np.float32).reshape(1, C)),
        "b1_r": np.ascontiguousarray(
            np.asarray(inputs["b1"], np.float32).reshape(DFF // 128, 128).T),
        "b2_r": np.ascontiguousarray(
            np.asarray(inputs["b2"], np.float32).reshape(NCT, 128).T),
        "lnw_r": np.ascontiguousarray(
            np.asarray(inputs["ln_w"], np.float32).reshape(NCT, 128).T),
        "lnw_row": np.ascontiguousarray(
            np.asarray(inputs["ln_w"], np.float32).reshape(1, C)),
        "ident_in": np.eye(128, dtype=np.float32),
        "ones_in": np.ones((128, 128), dtype=np.float32),
    }
    kk = np.arange(T, dtype=np.float32).reshape(NKT, 128)
    in_maps = []
    for c in range(N_CORES):
        b, m = divmod(c, 4)
        qq = np.arange(m * TCH, (m + 1) * TCH, dtype=np.float32)
        mask = (kk[:, :, None] <= qq[None, None, :]).astype(ml_dtypes.bfloat16)
        per = dict(shared)
        per["xct"] = np.ascontiguousarray(
            x[b, m * TCH:(m + 1) * TCH, :].T.astype(ml_dtypes.bfloat16))
        per["maskq"] = np.ascontiguousarray(mask)
        in_maps.append(per)
    return in_maps


def kernel(**inputs):
    from concourse.bass_utils import run_bass_kernel_spmd

    nc = _CACHE.get("nc")
    if nc is None:
        nc = _CACHE["nc"] = _build()
    in_maps = _stage(inputs)
    res = run_bass_kernel_spmd(nc, in_maps, core_ids=list(range(N_CORES)))
    out = np.empty((B, T, C), dtype=np.float32)
    for c in range(N_CORES):
        b, m = divmod(c, 4)
        out[b, m * TCH:(m + 1) * TCH, :] = res.results[c]["outc"]
    return out



# revision 22
# speedup vs baseline: 1.9895x; 1.9895x over previous
"""Trainium2 Bass kernel for a single decoder block (B=2, T=2048, C=1024,
NH=16, DFF=4096), distributed over 8 NeuronCores.

Sharding: token-parallel. Core c owns tokens [512*(c%4), 512*(c%4+1)) of
batch c//4. Each core computes q/k/v for its own tokens, AllGathers k and v
within its 4-core batch group, runs causal attention for all 16 heads over
its query chunk (full key range, causality enforced by multiplicative
masks), then LayerNorm residual + FFN + final LayerNorm residual locally.
The host reassembles the 8 output chunks.

Activations flow feature-major ([feature, token]) so matmul contractions
always have the contraction dim on SBUF partitions; fp32r (TF32-like) is
used for all matmul operands, fp32 elsewhere.
"""

import sys

if "/opt/trn_rl_repo" not in sys.path:
    sys.path.insert(0, "/opt/trn_rl_repo")

from contextlib import ExitStack

import numpy as np
import ml_dtypes

B, T, C = 2, 2048, 1024
NH, HD, DFF = 16, 64, 4096
N_CORES = 8
TCH = 512          # tokens per core
NCT = C // 128     # 8 feature tiles
NKT = T // 128     # 16 key tiles per batch
NPAIR = NH // 2    # 8 head pairs
SCALE = 1.0 / 32.0  # 1/sqrt(C)
EPS = 1e-5

_CACHE = {}


def _build(reps=1, collective=True):
    import concourse.mybir as mybir
    import concourse.tile as tile
    from concourse import bacc

    F32 = mybir.dt.float32
    F32R = mybir.dt.float32r
    BF16 = mybir.dt.bfloat16
    AF = mybir.ActivationFunctionType
    ALU = mybir.AluOpType

    nc = bacc.Bacc("TRN2", target_bir_lowering=False, debug=False,
                   num_devices=N_CORES)

    def din(name, shape):
        return nc.dram_tensor(name, shape, F32, kind="ExternalInput").ap()

    def dinb(name, shape):
        return nc.dram_tensor(name, shape, mybir.dt.bfloat16,
                              kind="ExternalInput").ap()

    xct = dinb("xct", [C, TCH])
    wq = dinb("wq", [C, C])
    wk = dinb("wk", [C, C])
    wv = dinb("wv", [C, C])
    w1 = dinb("w1", [C, DFF])
    w2 = dinb("w2", [DFF, C])
    bq_r = din("bq_r", [128, NCT])
    bk_r = din("bk_r", [128, NCT])
    bv_row = din("bv_row", [1, C])
    b1_r = din("b1_r", [128, DFF // 128])
    b2_r = din("b2_r", [128, NCT])
    lnw_r = din("lnw_r", [128, NCT])
    lnw_row = din("lnw_row", [1, C])
    ident_in = din("ident_in", [128, 128])
    ones_in = din("ones_in", [128, 128])
    maskq = nc.dram_tensor("maskq", [NKT, 128, TCH], mybir.dt.bfloat16,
                           kind="ExternalInput").ap()
    outc = nc.dram_tensor("outc", [TCH, C], F32, kind="ExternalOutput").ap()

    import concourse.bass as bass

    def bcast_row(row_ap, parts=128):
        # view a [1, N] DRAM row as [parts, N] with partition step 0
        return bass.AP(tensor=row_ap.tensor, offset=row_ap.offset,
                       ap=[[0, parts]] + list(row_ap.ap[1:]))

    with tile.TileContext(nc) as tc, ExitStack() as S0, \
            nc.allow_low_precision(reason="fp32r matmul operand rounding"):
      persist = S0.enter_context(tc.tile_pool(name="persist", bufs=1))
      dram = S0.enter_context(tc.tile_pool(name="dram", bufs=1, space="DRAM"))
      for _rep in range(reps):

          # ---- phase-1-critical constants only; the rest load later
          bk_sb = persist.tile([128, NCT], F32)
          nc.sync.dma_start(bk_sb, bk_r)
          bq_sb = persist.tile([128, NCT], F32)
          nc.sync.dma_start(bq_sb, bq_r)
          ones_all = persist.tile([128, 128], F32R)
          nc.sync.dma_start(ones_all, ones_in.bitcast(F32R))
          eps_sb = persist.tile([128, 1], F32)
          nc.vector.memset(eps_sb, EPS)
          ident_f = persist.tile([128, 128], F32)
          bv_sb = persist.tile([128, C], F32)
          lnwbc_sb = persist.tile([128, C], F32)
          b1_sb = persist.tile([128, DFF // 128], F32)
          b2_sb = persist.tile([128, NCT], F32)
          lnw_sb = persist.tile([128, NCT], F32)

          # ---- DRAM buffers for the k/v AllGathers (v first, k second)
          VROW = NH * (HD + 1)  # 1040: per-token v row with ones col per head
          kg_loc = dram.tile([C * TCH], BF16)
          vg_loc = dram.tile([TCH * VROW], BF16)
          kg_out = dram.tile([4, C * TCH], BF16)
          vg_out = dram.tile([4, TCH * VROW], BF16)
          k_loc = kg_loc[:].rearrange("(f t) -> f t", t=TCH)   # [1024, 512]
          v_loc = vg_loc[:].rearrange("(t x) -> t x", x=VROW)  # [512, 1040]

          with ExitStack() as SQA:
              qa_pool = SQA.enter_context(tc.tile_pool(name="qa", bufs=1))
              qT = qa_pool.tile([128, NPAIR, TCH], BF16)
              aT = qa_pool.tile([128, NCT, TCH], F32R)
              w1p = SQA.enter_context(tc.tile_pool(name="w1p", bufs=3))
              w2p = SQA.enter_context(tc.tile_pool(name="w2p", bufs=2))
              SATT = SQA.enter_context(ExitStack())
              mpool = SATT.enter_context(tc.tile_pool(name="mpool", bufs=1))
              vaugp = SATT.enter_context(tc.tile_pool(name="vaugp", bufs=1))
              kpool = SATT.enter_context(tc.tile_pool(name="kpool", bufs=6))
              masks_sb = mpool.tile([128, NKT, TCH], BF16)

              # ================= Phase 1: x^T, q/k/v projections ============
              with ExitStack() as S1:
                  xtp = S1.enter_context(tc.tile_pool(name="xtp", bufs=1))
                  wqk = S1.enter_context(tc.tile_pool(name="wqk", bufs=3))
                  wvp = S1.enter_context(tc.tile_pool(name="wvp", bufs=1))
                  kvsb = S1.enter_context(tc.tile_pool(name="kvsb", bufs=4))
                  vstgp = S1.enter_context(tc.tile_pool(name="vstgp", bufs=1))
                  qkps = S1.enter_context(tc.tile_pool(name="qkps", bufs=4, space="PSUM"))
                  vps = S1.enter_context(tc.tile_pool(name="vps", bufs=4, space="PSUM"))

                  xT = xtp.tile([128, NCT, TCH], BF16)
                  nc.sync.dma_start(
                      xT, xct.rearrange("(ci p) t -> p ci t", p=128))

                  # k^T first: it gates the k AllGather and S^T
                  for p in range(NCT):
                      wt = wqk.tile([128, NCT, 128], BF16, tag="wt")
                      nc.sync.dma_start(
                          wt, wk.rearrange("(ci r) f -> r ci f", r=128)
                          [:, :, p * 128:(p + 1) * 128])
                      ps = qkps.tile([128, TCH], F32, tag="qkp")
                      for ci in range(NCT):
                          nc.tensor.matmul(ps, wt[:, ci, :], xT[:, ci, :],
                                           start=(ci == 0), stop=(ci == NCT - 1))
                      ksb = kvsb.tile([128, TCH], BF16, tag="ksb")
                      nc.scalar.activation(ksb, ps, AF.Identity,
                                           bias=bk_sb[:, p:p + 1])
                      nc.sync.dma_start(k_loc[p * 128:(p + 1) * 128, :], ksb)
                  if collective:
                      nc.gpsimd.collective_compute(
                          "AllGather", mybir.AluOpType.bypass,
                          replica_groups=[[0, 1, 2, 3], [4, 5, 6, 7]],
                          ins=[kg_loc[:].opt()], outs=[kg_out[:].opt()])
                  else:
                      for g in range(4):
                          nc.sync.dma_start(kg_out[g, 0:1024], kg_loc[0:1024])

                  nc.gpsimd.dma_start(bv_sb, bcast_row(bv_row))

                  # v: token-major, written directly in v_aug layout
                  # [512 tok, 16 head, 64+1] with a ones column per head
                  vstg = [vstgp.tile([128, NH, HD + 1], BF16, name=f"vstg{tt}")
                          for tt in range(4)]
                  for tt in range(4):
                      nc.vector.memset(vstg[tt][:, :, HD:HD + 1], 1.0)
                  for fvt in range(2):
                      wt = wvp.tile([128, NCT, TCH], BF16, tag="wtv")
                      nc.sync.dma_start(
                          wt, wv.rearrange("(ci r) f -> r ci f", r=128)
                          [:, :, fvt * TCH:(fvt + 1) * TCH])
                      pss = [vps.tile([128, TCH], F32, tag="vp", name=f"vp{fvt}_{i}")
                             for i in range(4)]
                      for ci in range(NCT):
                          for tt in range(4):
                              nc.tensor.matmul(
                                  pss[tt], xT[:, ci, tt * 128:(tt + 1) * 128],
                                  wt[:, ci, :],
                                  start=(ci == 0), stop=(ci == NCT - 1))
                      bvv = bv_sb[:, fvt * TCH:(fvt + 1) * TCH].rearrange(
                          "p (h d) -> p h d", d=HD)
                      for tt in range(4):
                          nc.vector.tensor_add(
                              vstg[tt][:, 8 * fvt:8 * fvt + 8, 0:HD],
                              pss[tt][:, :].rearrange("p (h d) -> p h d", d=HD),
                              bvv)
                  for tt in range(4):
                      nc.sync.dma_start(
                          v_loc[tt * 128:(tt + 1) * 128, :],
                          vstg[tt][:, :, :])
                  if collective:
                      nc.gpsimd.collective_compute(
                          "AllGather", mybir.AluOpType.bypass,
                          replica_groups=[[0, 1, 2, 3], [4, 5, 6, 7]],
                          ins=[vg_loc[:].opt()], outs=[vg_out[:].opt()])
                  else:
                      for g in range(4):
                          nc.sync.dma_start(vg_out[g, 0:1024], vg_loc[0:1024])

                  nc.sync.dma_start(
                      masks_sb, maskq.rearrange("k p t -> p k t"))
                  nc.sync.dma_start(ident_f, ident_in)
                  nc.gpsimd.dma_start(lnwbc_sb, bcast_row(lnw_row))
                  nc.sync.dma_start(b1_sb, b1_r)
                  nc.sync.dma_start(b2_sb, b2_r)
                  nc.sync.dma_start(lnw_sb, lnw_r)

                  # q^T: [1024 feat, 512 tok], feature-major
                  for p in range(NCT):
                      wt = wqk.tile([128, NCT, 128], BF16, tag="wt")
                      nc.sync.dma_start(
                          wt, wq.rearrange("(ci r) f -> r ci f", r=128)
                          [:, :, p * 128:(p + 1) * 128])
                      ps = qkps.tile([128, TCH], F32, tag="qkp")
                      for ci in range(NCT):
                          nc.tensor.matmul(ps, wt[:, ci, :], xT[:, ci, :],
                                           start=(ci == 0), stop=(ci == NCT - 1))
                      nc.scalar.activation(qT[:, p, :], ps, AF.Identity,
                                           bias=bq_sb[:, p:p + 1])

              # ================= Phase 3: attention =========================
              with ExitStack() as S3:
                  pup = S3.enter_context(tc.tile_pool(name="pup", bufs=4))
                  dnp = S3.enter_context(tc.tile_pool(name="dnp", bufs=3))
                  stps = S3.enter_context(tc.tile_pool(name="stps", bufs=2, space="PSUM"))
                  avps = S3.enter_context(tc.tile_pool(name="avps", bufs=3, space="PSUM"))
                  rdps = S3.enter_context(tc.tile_pool(name="rdps", bufs=1, space="PSUM"))

                  # v_aug[j, head, 0:64] = v_head, [.., 64] = 1.0 (denominator)
                  vaug = vaugp.tile([128, NKT, NH, HD + 1], BF16)
                  for jt in range(NKT):
                      r, u = divmod(jt, 4)
                      vsrc = vg_out[r].rearrange("(t x) -> t x", x=VROW)
                      nc.sync.dma_start(
                          vaug[:, jt, :, :].rearrange("p h d -> p (h d)"),
                          vsrc[u * 128:(u + 1) * 128, :])

                  for p in range(NPAIR):
                      avA = avps.tile([HD + 1, TCH], F32, tag="av")
                      avB = avps.tile([HD + 1, TCH], F32, tag="av")
                      ktile = None
                      for kt in range(NKT):
                          r, u = divmod(kt, 4)
                          if u == 0:
                              ktile = kpool.tile([128, TCH], BF16, tag="kt")
                              ksrc = kg_out[r].rearrange("(f t) -> f t", t=TCH)
                              nc.sync.dma_start(
                                  ktile, ksrc[p * 128:(p + 1) * 128, :])
                          st = stps.tile([128, 2, TCH], F32, tag="st")
                          nc.tensor.matmul(st[:, 0, :],
                                           ktile[0:64, u * 128:(u + 1) * 128],
                                           qT[0:64, p, :], start=True, stop=True)
                          nc.tensor.matmul(st[:, 1, :],
                                           ktile[64:128, u * 128:(u + 1) * 128],
                                           qT[64:128, p, :], start=True, stop=True)
                          pu = pup.tile([128, 2, TCH], BF16, tag="pu")
                          nc.scalar.activation(pu[:], st[:], AF.Exp, scale=SCALE)
                          m = masks_sb[:, kt, :]
                          m2 = bass.AP(tensor=m.tensor, offset=m.offset,
                                       ap=[list(m.ap[0]), [0, 2],
                                           list(m.ap[1])])
                          nc.vector.tensor_mul(pu[:, :, :], pu[:, :, :], m2)
                          nc.tensor.matmul(avA, vaug[:, kt, 2 * p, :],
                                           pu[:, 0, :],
                                           start=(kt == 0), stop=(kt == NKT - 1))
                          nc.tensor.matmul(avB, vaug[:, kt, 2 * p + 1, :],
                                           pu[:, 1, :],
                                           start=(kt == 0), stop=(kt == NKT - 1))
                      # normalize: a_head = av[0:64] / av[64]
                      for av, odd in ((avA, 0), (avB, 1)):
                          den = dnp.tile([128, TCH], F32R, tag="den")
                          nc.vector.tensor_copy(den[64:65, :], av[64:65, :])
                          rd = rdps.tile([64, TCH], F32, tag="rd")
                          nc.tensor.matmul(rd, ones_all[64:65, 0:64],
                                           den[64:65, :], start=True, stop=True)
                          rdsb = dnp.tile([64, TCH], F32, tag="rdsb")
                          nc.vector.reciprocal(rdsb[:], rd[:])
                          if not odd:
                              nc.vector.tensor_mul(aT[0:64, p, :], av[0:64, :],
                                                   rdsb[:])
                          else:
                              tmp = dnp.tile([64, TCH], F32R, tag="tmp")
                              nc.vector.tensor_mul(tmp[:], av[0:64, :], rdsb[:])
                              nc.sync.dma_start(aT[64:128, p, :], tmp[:])

              SATT.close()

              # ================= Phase 4: h = a + LN(a), feature-major ======
              with ExitStack() as SH:
                  hp = SH.enter_context(tc.tile_pool(name="hp", bufs=1))
                  hT = hp.tile([128, NCT, TCH], BF16)
                  fT = hp.tile([128, NCT, TCH], F32)

                  with ExitStack() as S4:
                      sqp = S4.enter_context(tc.tile_pool(name="sqp", bufs=3))
                      stsb = S4.enter_context(tc.tile_pool(name="stsb", bufs=1))
                      smps = S4.enter_context(tc.tile_pool(name="smps", bufs=2, space="PSUM"))
                      bcps = S4.enter_context(tc.tile_pool(name="bcps", bufs=2, space="PSUM"))

                      sum_ps = smps.tile([1, TCH], F32, tag="sm")
                      sq_ps = smps.tile([1, TCH], F32, tag="sm")
                      for ci in range(NCT):
                          nc.tensor.matmul(sum_ps, ones_all[:, 0:1], aT[:, ci, :],
                                           start=(ci == 0), stop=(ci == NCT - 1))
                      for ci in range(NCT):
                          asq = sqp.tile([128, TCH], F32R, tag="asq")
                          nc.scalar.activation(asq, aT[:, ci, :], AF.Square)
                          nc.tensor.matmul(sq_ps, ones_all[:, 0:1], asq[:],
                                           start=(ci == 0), stop=(ci == NCT - 1))
                      mu_sb = stsb.tile([1, TCH], F32R, tag="s1")
                      nc.vector.tensor_scalar_mul(mu_sb, sum_ps, 1.0 / C)
                      ex2 = stsb.tile([1, TCH], F32, tag="s2")
                      nc.vector.tensor_scalar_mul(ex2, sq_ps, 1.0 / C)
                      musq = stsb.tile([1, TCH], F32, tag="s3")
                      nc.vector.tensor_mul(musq, mu_sb, mu_sb)
                      var = stsb.tile([1, TCH], F32, tag="s4")
                      nc.vector.tensor_sub(var, ex2, musq)
                      sd = stsb.tile([1, TCH], F32, tag="s5")
                      nc.scalar.activation(sd, var, AF.Sqrt, bias=eps_sb[0:1, :])
                      rs_sb = stsb.tile([1, TCH], F32R, tag="s6")
                      nc.vector.reciprocal(rs_sb, sd)
                      mu_bc = bcps.tile([128, TCH], F32, tag="bc")
                      nc.tensor.matmul(mu_bc, ones_all[0:1, :], mu_sb[:],
                                       start=True, stop=True)
                      rs_bc = bcps.tile([128, TCH], F32, tag="bc")
                      nc.tensor.matmul(rs_bc, ones_all[0:1, :], rs_sb[:],
                                       start=True, stop=True)
                      for ci in range(NCT):
                          t1 = sqp.tile([128, TCH], F32, tag="t1")
                          nc.vector.tensor_sub(t1, aT[:, ci, :], mu_bc)
                          t2 = sqp.tile([128, TCH], F32, tag="t2")
                          nc.vector.tensor_mul(t2, t1, rs_bc)
                          nc.vector.scalar_tensor_tensor(
                              out=hT[:, ci, :], in0=t2,
                              scalar=lnw_sb[:, ci:ci + 1], in1=aT[:, ci, :],
                              op0=ALU.mult, op1=ALU.add)

                  # ================= Phase 5/6: FFN =========================
                  with ExitStack() as S5:
                      gp = S5.enter_context(tc.tile_pool(name="gp", bufs=1))
                      ffps = S5.enter_context(tc.tile_pool(name="ffps", bufs=4, space="PSUM"))

                      gT = gp.tile([128, DFF // 128, TCH], BF16)
                      for mt in range(DFF // 128):
                          wt = w1p.tile([128, NCT, 128], BF16, tag="w1t")
                          nc.sync.dma_start(
                              wt, w1.rearrange("(ci r) f -> r ci f", r=128)
                              [:, :, mt * 128:(mt + 1) * 128])
                          ps = ffps.tile([128, TCH], F32, tag="f1")
                          for ci in range(NCT):
                              nc.tensor.matmul(ps, wt[:, ci, :], hT[:, ci, :],
                                               start=(ci == 0), stop=(ci == NCT - 1))
                          nc.scalar.activation(gT[:, mt, :], ps, AF.Relu,
                                               bias=b1_sb[:, mt:mt + 1])
                      for ci in range(NCT):
                          wt = w2p.tile([128, DFF // 128, 128], BF16, tag="w2t")
                          nc.sync.dma_start(
                              wt, w2.rearrange("(gk r) f -> r gk f", r=128)
                              [:, :, ci * 128:(ci + 1) * 128])
                          ps = ffps.tile([128, TCH], F32, tag="f2")
                          for gk in range(DFF // 128):
                              nc.tensor.matmul(ps, wt[:, gk, :], gT[:, gk, :],
                                               start=(gk == 0),
                                               stop=(gk == DFF // 128 - 1))
                          nc.scalar.activation(fT[:, ci, :], ps, AF.Identity,
                                               bias=b2_sb[:, ci:ci + 1])

                  # ================= Phase 7: out = f + LN(f), token-major ==
                  with ExitStack() as S7:
                      op7 = S7.enter_context(tc.tile_pool(name="op7", bufs=2))
                      tp7 = S7.enter_context(tc.tile_pool(name="tp7", bufs=2, space="PSUM"))

                      for tt in range(4):
                          ftok = op7.tile([128, C], F32, tag="ftok")
                          for ci in range(NCT):
                              tp = tp7.tile([128, 128], F32, tag="tp")
                              nc.tensor.transpose(
                                  tp, fT[:, ci, tt * 128:(tt + 1) * 128],
                                  ident_f[:])
                              nc.vector.tensor_copy(
                                  ftok[:, ci * 128:(ci + 1) * 128], tp[:])
                          stats = op7.tile([128, 2, nc.vector.BN_STATS_DIM],
                                           F32, tag="bst")
                          nc.vector.bn_stats(stats[:, 0], ftok[:, 0:512])
                          nc.vector.bn_stats(stats[:, 1], ftok[:, 512:1024])
                          mv = op7.tile([128, nc.vector.BN_AGGR_DIM], F32,
                                        tag="mv")
                          nc.vector.bn_aggr(mv, stats)
                          rs7 = op7.tile([128, 1], F32, tag="rs7")
                          nc.scalar.activation(rs7, mv[:, 1:2], AF.Sqrt,
                                               bias=eps_sb)
                          nc.vector.reciprocal(rs7, rs7)
                          t1 = op7.tile([128, C], F32, tag="t17")
                          nc.vector.tensor_scalar(
                              out=t1, in0=ftok, scalar1=mv[:, 0:1], scalar2=rs7,
                              op0=ALU.subtract, op1=ALU.mult)
                          nc.vector.tensor_mul(t1, t1, lnwbc_sb)
                          otok = op7.tile([128, C], F32, tag="otok")
                          nc.vector.tensor_add(otok, t1, ftok)
                          nc.sync.dma_start(outc[tt * 128:(tt + 1) * 128, :],
                                            otok)

    nc.compile()
    return nc


def _stage(inputs):
    x = np.ascontiguousarray(np.asarray(inputs["x"], dtype=np.float32))
    bf = ml_dtypes.bfloat16
    shared = {
        "wq": np.ascontiguousarray(np.asarray(inputs["Wq"], np.float32).astype(bf)),
        "wk": np.ascontiguousarray(np.asarray(inputs["Wk"], np.float32).astype(bf)),
        "wv": np.ascontiguousarray(np.asarray(inputs["Wv"], np.float32).astype(bf)),
        "w1": np.ascontiguousarray(np.asarray(inputs["W1"], np.float32).astype(bf)),
        "w2": np.ascontiguousarray(np.asarray(inputs["W2"], np.float32).astype(bf)),
        "bq_r": np.ascontiguousarray(
            np.asarray(inputs["bq"], np.float32).reshape(NCT, 128).T),
        "bk_r": np.ascontiguousarray(
            np.asarray(inputs["bk"], np.float32).reshape(NCT, 128).T),
        "bv_row": np.ascontiguousarray(
            np.asarray(inputs["bv"], np.float32).reshape(1, C)),
        "b1_r": np.ascontiguousarray(
            np.asarray(inputs["b1"], np.float32).reshape(DFF // 128, 128).T),
        "b2_r": np.ascontiguousarray(
            np.asarray(inputs["b2"], np.float32).reshape(NCT, 128).T),
        "lnw_r": np.ascontiguousarray(
            np.asarray(inputs["ln_w"], np.float32).reshape(NCT, 128).T),
        "lnw_row": np.ascontiguousarray(
            np.asarray(inputs["ln_w"], np.float32).reshape(1, C)),
        "ident_in": np.eye(128, dtype=np.float32),
        "ones_in": np.ones((128, 128), dtype=np.float32),
    }
    kk = np.arange(T, dtype=np.float32).reshape(NKT, 128)
    in_maps = []
    for c in range(N_CORES):
        b, m = divmod(c, 4)
        qq = np.arange(m * TCH, (m + 1) * TCH, dtype=np.float32)
        mask = (kk[:, :, None] <= qq[None, None, :]).astype(ml_dtypes.bfloat16)
        per = dict(shared)
        per["xct"] = np.ascontiguousarray(
            x[b, m * TCH:(m + 1) * TCH, :].T.astype(ml_dtypes.bfloat16))
        per["maskq"] = np.ascontiguousarray(mask)
        in_maps.append(per)
    return in_maps


def kernel(**inputs):
    from concourse.bass_utils import run_bass_kernel_spmd

    nc = _CACHE.get("nc")
    if nc is None:
        nc = _CACHE["nc"] = _build()
    in_maps = _stage(inputs)
    res = run_bass_kernel_spmd(nc, in_maps, core_ids=list(range(N_CORES)))
    out = np.empty((B, T, C), dtype=np.float32)
    for c in range(N_CORES):
        b, m = divmod(c, 4)
        out[b, m * TCH:(m + 1) * TCH, :] = res.results[c]["outc"]
    return out

